# revision 1
# baseline (speedup 1.0000x reference)
"""Trainium2 Bass kernel for batched differentiable-Markowitz layer.

Solves, for each of 2048 rows p:  min_w 0.5 w'Sigma w + p'w  s.t. w in simplex,
matching a 200-step FISTA reference. Key structure:

  * FISTA's fixed point is independent of lr and the momentum schedule, so lr
    comes from an on-device power-iteration bound on ||Sigma||_2.
  * The reference reaches the fp32 noise floor in ~45 steps; we run 46 in a
    precision cascade (28 bf16 / 10 float32r / 8 fp32 matmul steps).  The
    cheap phases only need an approximate iterate; the exact fp32 tail
    polishes to the fp32 fixed point (contraction ~0.7/step).
  * Per step: W = w@A accumulates in PSUM (A = I - lr*Sigma);
    v = (1+c)W - (c*W_prev + lr*p) and the next step's u are single fused
    scalar_tensor_tensor ops over a merged [128,512] view of both batch
    tiles; the simplex projection threshold theta is warm-started with one
    Newton update per step (relu+rowsum fused on ACT activation accum,
    active-count lagged and refreshed every 3rd step).
  * w is transposed on the PE (per-phase dtype identities) to form the next
    step's matmul weights.

Sharding: data-parallel over the batch, 256 rows per core, Sigma replicated,
no collectives.
"""

import math
from contextlib import ExitStack

import numpy as np

import concourse.bass as bass  # noqa: F401
import concourse.tile as tile
from concourse import bacc, mybir
from concourse.bass_utils import run_bass_kernel_spmd

F32 = mybir.dt.float32
F32R = mybir.dt.float32r
BF16 = mybir.dt.bfloat16
OP = mybir.AluOpType
RELU = mybir.ActivationFunctionType.Relu
COPY = mybir.ActivationFunctionType.Copy

N = 256           # problem dimension
B_CORE = 256      # batch rows per core
N_CORES = 8
NB = B_CORE // 128
NK = N // 128
NBW = NB * N      # merged free width (both batch tiles side by side)

N_BF = 16         # bf16 matmul steps
N_MID = 12        # float32r matmul steps
N_POLISH = 10     # exact fp32 matmul steps
K0_NEWTON = 4     # cold-start Newton iterations (step 0)
POW_ITERS = 5
L_SAFETY = 1.10
CNT_EVERY = 4     # refresh lagged 1/cnt every k-th step


def _momentum_coeffs(n):
    t = np.float32(1.0)
    cs = []
    for _ in range(n + 3):
        t_next = np.float32(0.5 * (1.0 + math.sqrt(1.0 + 4.0 * float(t) * float(t))))
        cs.append(float((t - np.float32(1.0)) / t_next))
        t = t_next
    return cs


def _make_identity(nc, ap, base=0):
    nc.gpsimd.memset(ap, 0.0)
    nc.gpsimd.affine_select(
        out=ap, in_=ap, compare_op=OP.not_equal, fill=1.0, base=base,
        pattern=[[-1, ap.shape[1]]], channel_multiplier=1)


def markowitz_tile_kernel(tc, out_w, in_p, in_sig, *,
                          n_bf=N_BF, n_mid=N_MID, n_polish=N_POLISH,
                          k0=K0_NEWTON, pow_iters=POW_ITERS, safety=L_SAFETY):
    nc = tc.nc
    ctx = ExitStack()
    n_steps = n_bf + n_mid + n_polish
    cs = _momentum_coeffs(n_steps)

    def phase_dt(t):
        if t < n_bf:
            return BF16
        if t < n_bf + n_mid:
            return F32R
        return F32

    const = ctx.enter_context(tc.tile_pool(name="const", bufs=1))
    vpool = ctx.enter_context(tc.tile_pool(name="v", bufs=5))
    upool = ctx.enter_context(tc.tile_pool(name="u", bufs=5))
    wpool = ctx.enter_context(tc.tile_pool(name="w", bufs=6))
    rpool = ctx.enter_context(tc.tile_pool(name="r", bufs=6))
    wtpool = ctx.enter_context(tc.tile_pool(name="wt", bufs=6))
    xtpool = ctx.enter_context(tc.tile_pool(name="xt", bufs=4))
    ps_w = ctx.enter_context(tc.tile_pool(name="psw", bufs=3, space="PSUM"))
    ps_t = ctx.enter_context(tc.tile_pool(name="pst", bufs=3, space="PSUM"))
    ps_m = ctx.enter_context(tc.tile_pool(name="psm", bufs=2, space="PSUM"))

    with ctx:
        # ---- persistent state ----
        S = [const.tile([128, N], F32, name=f"S{k}") for k in range(NK)]
        P = const.tile([128, NBW], F32, name="P")     # lr*p, both tiles merged
        A = [const.tile([128, N], F32, name=f"A{k}") for k in range(NK)]
        A_r = [const.tile([128, N], F32R, name=f"Ar{k}") for k in range(NK)]
        A_b = [const.tile([128, N], BF16, name=f"Ab{k}") for k in range(NK)]
        IA = [const.tile([128, N], F32, name=f"IA{k}") for k in range(NK)]
        ID = const.tile([128, 128], F32, name="ID")
        ID_r = const.tile([128, 128], F32R, name="IDr")
        ID_b = const.tile([128, 128], BF16, name="IDb")
        ONES = const.tile([128, 1], F32, name="ONES")
        th = [const.tile([128, 1], F32, name=f"th{b}")[:] for b in range(NB)]
        sv = [const.tile([128, 1], F32, name=f"sv{b}")[:] for b in range(NB)]
        cv = [const.tile([128, 1], F32, name=f"cv{b}")[:] for b in range(NB)]
        cc = [const.tile([128, 1], F32, name=f"cc{b}")[:] for b in range(NB)]
        ic = [const.tile([128, 1], F32, name=f"ic{b}")[:] for b in range(NB)]
        dl = [const.tile([128, 1], F32, name=f"dl{b}")[:] for b in range(NB)]
        lr_vec = const.tile([128, 1], F32, name="lrv")
        nlr_vec = const.tile([128, 1], F32, name="nlrv")
        ray = const.tile([1, 128], F32, name="ray")
        ray_i = const.tile([1, 128], F32, name="rayi")
        lmax = const.tile([1, 1], F32, name="lmax")
        lsafe = const.tile([1, 1], F32, name="lsafe")
        lr_s = const.tile([1, 1], F32, name="lrs")
        nlr_s = const.tile([1, 1], F32, name="nlrs")
        w0f = const.tile([128, N], F32, name="w0f")

        # ---- load inputs ----
        for k in range(NK):
            nc.sync.dma_start(S[k][:], in_sig[128 * k:128 * (k + 1), :])
        for b in range(NB):
            nc.sync.dma_start(P[:, N * b:N * (b + 1)],
                              in_p[128 * b:128 * (b + 1), :])

        # ---- constants ----
        _make_identity(nc, ID[:])
        nc.vector.tensor_copy(ID_r[:], ID[:])
        nc.vector.tensor_copy(ID_b[:], ID[:])
        for k in range(NK):
            _make_identity(nc, IA[k][:], base=128 * k)
        nc.gpsimd.memset(ONES[:], 1.0)
        nc.gpsimd.memset(w0f[:], 1.0 / N)

        # ---- power iteration for L (bf16, transposed layout) ----
        S_b = [const.tile([128, N], BF16, name=f"Sb{k}") for k in range(NK)]
        for k in range(NK):
            nc.vector.tensor_copy(S_b[k][:], S[k][:])
        xc = [S_b[k][:, 0:128] for k in range(NK)]
        xp = None
        for it in range(pow_iters):
            xn = []
            for j in range(NK):
                px = ps_m.tile([128, 128], F32, tag="pps", name="pps")
                for k in range(NK):
                    nc.tensor.matmul(px[:], S_b[k][:, 128 * j:128 * (j + 1)],
                                     xc[k],
                                     start=(k == 0), stop=(k == NK - 1))
                xs = xtpool.tile([128, 128], BF16, tag="xs", name="xs")
                nc.scalar.copy(xs[:], px[:])
                xn.append(xs)
            xp, xc = xc, [t[:] for t in xn]
        pnum = ps_m.tile([1, 128], F32, tag="pps", name="pps")
        pden = ps_m.tile([1, 128], F32, tag="pps", name="pps")
        for k in range(NK):
            prod_n = xtpool.tile([128, 128], F32, tag="prodn", name="prodn")
            prod_d = xtpool.tile([128, 128], F32, tag="prodd", name="prodd")
            nc.vector.tensor_tensor(prod_n[:], xc[k], xc[k], OP.mult)
            nc.vector.tensor_tensor(prod_d[:], xp[k], xc[k], OP.mult)
            nc.tensor.matmul(pnum[:], ONES[:], prod_n[:],
                             start=(k == 0), stop=(k == NK - 1))
            nc.tensor.matmul(pden[:], ONES[:], prod_d[:],
                             start=(k == 0), stop=(k == NK - 1))
        nc.vector.reciprocal(ray_i[:], pden[:])
        nc.vector.tensor_tensor(ray[:], pnum[:], ray_i[:], OP.mult)
        nc.vector.tensor_reduce(lmax[:], ray[:], axis=mybir.AxisListType.X, op=OP.max)
        nc.vector.tensor_scalar(lsafe[:], lmax[:], float(safety), None, OP.mult)
        nc.vector.reciprocal(lr_s[:], lsafe[:])
        nc.vector.tensor_scalar(nlr_s[:], lr_s[:], -1.0, None, OP.mult)
        nc.gpsimd.partition_broadcast(lr_vec[:], lr_s[:])
        nc.gpsimd.partition_broadcast(nlr_vec[:], nlr_s[:])

        # ---- A = I - lr*Sigma (+casts);  P <- lr*p ----
        for k in range(NK):
            nc.vector.scalar_tensor_tensor(A[k][:], S[k][:], nlr_vec[:, 0:1],
                                           IA[k][:], op0=OP.mult, op1=OP.add)
            nc.vector.tensor_copy(A_r[k][:], A[k][:])
            nc.vector.tensor_copy(A_b[k][:], A[k][:])
        nc.vector.tensor_scalar(P[:], P[:], lr_vec[:, 0:1], None, OP.mult)

        # ---- initial weights: w0 = 1/N (transpose-invariant) ----
        wta = []
        for b in range(NB):
            a0 = wtpool.tile([128, N], phase_dt(0), tag=f"wta{b}", name=f"wta{b}")
            nc.vector.tensor_copy(a0[:], w0f[:])
            wta.append(a0)

        u_prev = [None] * NB     # step0 uses lr*p directly
        w_cur = [None] * NB

        def tile_step(b, t):
            c = cs[t]
            Amm = {BF16: A_b, F32R: A_r, F32: A}[phase_dt(t)]
            # W = w@A in PSUM; v = (1+c)W - u; u_next = c'W + lr*p
            pw = ps_w.tile([128, N], F32, tag="psW", name="psW")
            for k in range(NK):
                nc.tensor.matmul(pw[:], wta[b][:, 128 * k:128 * (k + 1)],
                                 Amm[k][:],
                                 start=(k == 0), stop=(k == NK - 1))
            v = vpool.tile([128, N], BF16 if t < n_bf else F32,
                           tag="v", name="v")
            u_in = P[:, N * b:N * (b + 1)] if t == 0 else u_prev[b][:]
            nc.vector.scalar_tensor_tensor(v[:], pw[:], 1.0 + c, u_in,
                                           op0=OP.mult, op1=OP.subtract)
            if t < n_steps - 1:
                un = upool.tile([128, N], BF16 if t + 1 < n_bf else F32,
                                tag="u", name="u")
                nc.vector.scalar_tensor_tensor(
                    un[:], pw[:], cs[t + 1], P[:, N * b:N * (b + 1)],
                    op0=OP.mult, op1=OP.add)
                u_prev[b] = un
            vb = v[:]

            # ---- projection (theta stored negated; bias adds) ----
            r = rpool.tile([128, N], F32, tag="r", name="r")
            nc.scalar.activation(r[:], vb, RELU,
                                 bias=th[b], accum_out=sv[b])
            nc.vector.scalar_tensor_tensor(dl[b], sv[b], 1.0, ic[b],
                                           op0=OP.subtract, op1=OP.mult)
            nc.vector.tensor_tensor(th[b], th[b], dl[b], OP.subtract)

            # ---- w = relu(v + ntheta) ----
            dt_n = phase_dt(t + 1)
            w = wpool.tile([128, N], dt_n, tag="w", name="w")
            if b == 0 and t > 0:
                nc.scalar.activation(w[:], vb, RELU, bias=th[b])
            else:
                nc.vector.tensor_scalar(w[:], vb, th[b], 0.0, OP.add, OP.max)
            w_cur[b] = w

            if t == n_steps - 1:
                nc.sync.dma_start(out_w[128 * b:128 * (b + 1), :], w[:])
                return

            if t % CNT_EVERY == 0:
                m = rpool.tile([128, N], F32, tag="m", name="m")
                nc.vector.tensor_scalar(m[:], w[:], 0.0, None,
                                        OP.is_gt, OP.add, accum_out=cv[b])
                nc.vector.tensor_scalar(cc[b], cv[b], 1.0, None, OP.max)
                nc.vector.reciprocal(ic[b], cc[b])

            # ---- next-step weights: wT (PE transpose + split copies) ----
            nwa = wtpool.tile([128, N], dt_n, tag=f"wta{b}", name=f"wta{b}")
            IDmm = {BF16: ID_b, F32R: ID_r, F32: ID}[dt_n]
            pt = ps_t.tile([128, N], dt_n, tag="psT", name="psT")
            for k in range(NK):
                sl = slice(128 * k, 128 * (k + 1))
                nc.tensor.transpose(pt[:, sl], w[:, sl], IDmm[:])
                if b == 0:
                    nc.scalar.copy(nwa[:, sl], pt[:, sl])
                else:
                    nc.vector.tensor_copy(nwa[:, sl], pt[:, sl])
            wta[b] = nwa

        def cold_start():
            # Step 0 for BOTH tiles with the k0 Newton iterations interleaved
            # so the two serial chains overlap on ACT/DVE.
            vbs = []
            for b in range(NB):
                pw = ps_w.tile([128, N], F32, tag="psW", name="psW")
                for k in range(NK):
                    nc.tensor.matmul(pw[:], wta[b][:, 128 * k:128 * (k + 1)],
                                     A_b[k][:],
                                     start=(k == 0), stop=(k == NK - 1))
                v = vpool.tile([128, N], BF16 if 0 < n_bf else F32,
                               tag="v", name="v")
                nc.vector.scalar_tensor_tensor(
                    v[:], pw[:], 1.0 + cs[0], P[:, N * b:N * (b + 1)],
                    op0=OP.mult, op1=OP.subtract)
                un = upool.tile([128, N], BF16 if 1 < n_bf else F32,
                                tag="u", name="u")
                nc.vector.scalar_tensor_tensor(
                    un[:], pw[:], cs[1], P[:, N * b:N * (b + 1)],
                    op0=OP.mult, op1=OP.add)
                u_prev[b] = un
                vbs.append(v[:])
            for b in range(NB):
                scr = rpool.tile([128, N], F32, tag="r", name="r")
                nc.scalar.activation(scr[:], vbs[b], COPY, accum_out=sv[b])
                nc.vector.tensor_scalar(th[b], sv[b], 1.0, -1.0 / N,
                                        OP.subtract, OP.mult)
            for it in range(k0):
                for b in range(NB):
                    r = rpool.tile([128, N], F32, tag="r", name="r")
                    nc.scalar.activation(r[:], vbs[b], RELU,
                                         bias=th[b], accum_out=sv[b])
                    m = rpool.tile([128, N], F32, tag="m", name="m")
                    nc.vector.tensor_scalar(m[:], r[:], 0.0, None,
                                            OP.is_gt, OP.add, accum_out=cv[b])
                for b in range(NB):
                    nc.vector.tensor_scalar(cc[b], cv[b], 1.0, None, OP.max)
                    nc.vector.reciprocal(ic[b], cc[b])
                    nc.vector.scalar_tensor_tensor(dl[b], sv[b], 1.0, ic[b],
                                                   op0=OP.subtract, op1=OP.mult)
                    nc.vector.tensor_tensor(th[b], th[b], dl[b], OP.subtract)
            dt_n = phase_dt(1)
            IDmm = {BF16: ID_b, F32R: ID_r, F32: ID}[dt_n]
            for b in range(NB):
                w = wpool.tile([128, N], dt_n, tag="w", name="w")
                nc.vector.tensor_scalar(w[:], vbs[b], th[b], 0.0, OP.add, OP.max)
                w_cur[b] = w
                m = rpool.tile([128, N], F32, tag="m", name="m")
                nc.vector.tensor_scalar(m[:], w[:], 0.0, None,
                                        OP.is_gt, OP.add, accum_out=cv[b])
                nc.vector.tensor_scalar(cc[b], cv[b], 1.0, None, OP.max)
                nc.vector.reciprocal(ic[b], cc[b])
                nwa = wtpool.tile([128, N], dt_n, tag=f"wta{b}", name=f"wta{b}")
                pt = ps_t.tile([128, N], dt_n, tag="psT", name="psT")
                for k in range(NK):
                    sl = slice(128 * k, 128 * (k + 1))
                    nc.tensor.transpose(pt[:, sl], w[:, sl], IDmm[:])
                    if b == 0:
                        nc.scalar.copy(nwa[:, sl], pt[:, sl])
                    else:
                        nc.vector.tensor_copy(nwa[:, sl], pt[:, sl])
                wta[b] = nwa

        # software-skewed emission: tile 1 runs one step behind tile 0.
        # Emit the older (ready) tile-1 step first so engines' FIFO order
        # lets it fill the stalls of tile 0's fresh chain.  Step 0 runs both
        # tiles jointly (interleaved cold start).
        cold_start()
        for t in range(1, n_steps + 1):
            if t >= 2:
                tile_step(1, t - 1)
            if t < n_steps:
                tile_step(0, t)


def build_nc(**kw):
    nc = bacc.Bacc("TRN2", target_bir_lowering=False, debug=False,
                   enable_asserts=False)
    p_in = nc.dram_tensor("p", [B_CORE, N], F32, kind="ExternalInput")
    s_in = nc.dram_tensor("sigma", [N, N], F32, kind="ExternalInput")
    w_out = nc.dram_tensor("w", [B_CORE, N], F32, kind="ExternalOutput")
    with tile.TileContext(nc) as tc:
        markowitz_tile_kernel(tc, w_out.ap(), p_in.ap(), s_in.ap(), **kw)
    nc.compile()
    return nc


_NC_CACHE = {}


def kernel(p_batch: np.ndarray, Sigma: np.ndarray, **kw) -> np.ndarray:
    B = p_batch.shape[0]
    rows = B // N_CORES
    assert rows == B_CORE and Sigma.shape == (N, N)
    key = tuple(sorted(kw.items()))
    if key not in _NC_CACHE:
        _NC_CACHE[key] = build_nc(**kw)
    nc = _NC_CACHE[key]
    p32 = np.ascontiguousarray(p_batch, dtype=np.float32)
    s32 = np.ascontiguousarray(Sigma, dtype=np.float32)
    in_maps = [{"p": p32[i * rows:(i + 1) * rows], "sigma": s32}
               for i in range(N_CORES)]
    res = run_bass_kernel_spmd(nc, in_maps, core_ids=list(range(N_CORES)))
    out = np.concatenate([r["w"] for r in res.results], axis=0)
    return out.astype(p_batch.dtype, copy=False)



# revision 4
# speedup vs baseline: 2.5591x; 2.5591x over previous
"""Trainium2 Bass kernel for batched differentiable-Markowitz layer.

Solves, for each of 2048 rows p:  min_w 0.5 w'Sigma w + p'w  s.t. w in simplex,
matching a 200-step FISTA reference.  The fixed point is independent of lr and
the momentum schedule, so we run 12 accelerated steps (10 bf16 + 2 float32r)
with constant momentum beta=0.5 (near-optimal for this spectrum, kappa~8):

  * y-streaming: the matmul input is y_t directly; PSUM accumulates
    z = y@A - lr*p in one group (2 bf16 A-tile matmuls + 1 f32r
    identity-matmul folding the constant -lr*p), so no u/v elementwise
    stage exists at all.
  * projection: in-step Newton on the simplex threshold theta.  One ACT pass
    computes r = relu(z + th) with a free row-sum accumulator, two tiny DVE
    ops update th, a second ACT pass re-reads PSUM to form
    w = relu(z + th_new).  Active-set count refreshed every 4th step.
  * momentum on DVE (bf16 2x/4x modes): d = w - w_prev; y' = beta*d + w.
  * y' is transposed on the PE (per-phase dtype identities) into the next
    step's stationary operand; lr comes from a 5-iter on-device power
    iteration with a 1.10 safety factor.

Two batch chains of 128 rows run software-skewed (chain 1 one step behind)
with op-level interleaved emission so PE/ACT/DVE overlap across chains.

Sharding: data-parallel over the batch, 256 rows per core, Sigma replicated,
no collectives.
"""

from contextlib import ExitStack

import numpy as np

import concourse.bass as bass  # noqa: F401
import concourse.tile as tile
from concourse import bacc, mybir
from concourse.bass_utils import run_bass_kernel_spmd

F32 = mybir.dt.float32
F32R = mybir.dt.float32r
BF16 = mybir.dt.bfloat16
OP = mybir.AluOpType
RELU = mybir.ActivationFunctionType.Relu
COPY = mybir.ActivationFunctionType.Copy

N = 256           # problem dimension
B_CORE = 256      # batch rows per core
N_CORES = 8
NB = B_CORE // 128
NK = N // 128

N_BF = 10         # bf16 matmul steps
N_R = 2           # float32r tail steps
K0_NEWTON = 2     # cold-start Newton iterations (step 0)
POW_ITERS = 5
L_SAFETY = 1.10
CNT_EVERY = 4     # refresh lagged 1/cnt every k-th step
BETA = 0.5        # constant momentum


def _make_identity(nc, ap, base=0):
    nc.gpsimd.memset(ap, 0.0)
    nc.gpsimd.affine_select(
        out=ap, in_=ap, compare_op=OP.not_equal, fill=1.0, base=base,
        pattern=[[-1, ap.shape[1]]], channel_multiplier=1)


def markowitz_tile_kernel(tc, out_w, in_p, in_sig, *,
                          n_bf=N_BF, n_r=N_R, k0=K0_NEWTON,
                          pow_iters=POW_ITERS, safety=L_SAFETY, beta=BETA):
    nc = tc.nc
    ctx = ExitStack()
    n_steps = n_bf + n_r

    def mdt(t):          # matmul dtype of step t
        return BF16 if t < n_bf else F32R

    def edt(t):          # elementwise dtype of r/w at step t
        return BF16 if t < n_bf - 1 else F32

    const = ctx.enter_context(tc.tile_pool(name="const", bufs=1))
    rpool = ctx.enter_context(tc.tile_pool(name="r", bufs=4))
    wpool = ctx.enter_context(tc.tile_pool(name="w", bufs=6))
    dpool = ctx.enter_context(tc.tile_pool(name="d", bufs=4))
    ypool = ctx.enter_context(tc.tile_pool(name="y", bufs=4))
    wtpool = ctx.enter_context(tc.tile_pool(name="wt", bufs=4))
    mpool = ctx.enter_context(tc.tile_pool(name="m", bufs=2))
    xtpool = ctx.enter_context(tc.tile_pool(name="xt", bufs=4))
    ps_w = ctx.enter_context(tc.tile_pool(name="psw", bufs=3, space="PSUM"))
    ps_t = ctx.enter_context(tc.tile_pool(name="pst", bufs=3, space="PSUM"))
    ps_m = ctx.enter_context(tc.tile_pool(name="psm", bufs=2, space="PSUM"))

    with ctx:
        # ---- persistent state ----
        S = [const.tile([128, N], F32, name=f"S{k}") for k in range(NK)]
        P = [const.tile([128, N], F32, name=f"P{b}") for b in range(NB)]
        A = [const.tile([128, N], F32, name=f"A{k}") for k in range(NK)]
        A_r = [const.tile([128, N], F32R, name=f"Ar{k}") for k in range(NK)]
        A_b = [const.tile([128, N], BF16, name=f"Ab{k}") for k in range(NK)]
        C_r = [const.tile([128, N], F32R, name=f"Cr{b}") for b in range(NB)]
        IA = [const.tile([128, N], F32, name=f"IA{k}") for k in range(NK)]
        ID = const.tile([128, 128], F32, name="ID")
        ID_r = const.tile([128, 128], F32R, name="IDr")
        ID_b = const.tile([128, 128], BF16, name="IDb")
        ONES = const.tile([128, 1], F32, name="ONES")
        th = [const.tile([128, 1], F32, name=f"th{b}")[:] for b in range(NB)]
        sv = [const.tile([128, 1], F32, name=f"sv{b}")[:] for b in range(NB)]
        cv = [const.tile([128, 1], F32, name=f"cv{b}")[:] for b in range(NB)]
        cc = [const.tile([128, 1], F32, name=f"cc{b}")[:] for b in range(NB)]
        ic = [const.tile([128, 1], F32, name=f"ic{b}")[:] for b in range(NB)]
        dl = [const.tile([128, 1], F32, name=f"dl{b}")[:] for b in range(NB)]
        lr_vec = const.tile([128, 1], F32, name="lrv")
        nlr_vec = const.tile([128, 1], F32, name="nlrv")
        ray = const.tile([1, 128], F32, name="ray")
        ray_i = const.tile([1, 128], F32, name="rayi")
        lmax = const.tile([1, 1], F32, name="lmax")
        lsafe = const.tile([1, 1], F32, name="lsafe")
        lr_s = const.tile([1, 1], F32, name="lrs")
        nlr_s = const.tile([1, 1], F32, name="nlrs")

        # ---- load inputs ----
        for k in range(NK):
            nc.sync.dma_start(S[k][:], in_sig[128 * k:128 * (k + 1), :])
        for b in range(NB):
            nc.sync.dma_start(P[b][:], in_p[128 * b:128 * (b + 1), :])

        # ---- constants ----
        _make_identity(nc, ID[:])
        nc.vector.tensor_copy(ID_r[:], ID[:])
        nc.vector.tensor_copy(ID_b[:], ID[:])
        for k in range(NK):
            _make_identity(nc, IA[k][:], base=128 * k)
        nc.gpsimd.memset(ONES[:], 1.0)

        # ---- power iteration for L (bf16, transposed layout) ----
        S_b = [const.tile([128, N], BF16, name=f"Sb{k}") for k in range(NK)]
        for k in range(NK):
            nc.vector.tensor_copy(S_b[k][:], S[k][:])
        xc = [S_b[k][:, 0:128] for k in range(NK)]
        xp = None
        for it in range(pow_iters):
            xn = []
            for j in range(NK):
                px = ps_m.tile([128, 128], F32, tag="pps", name="pps")
                for k in range(NK):
                    nc.tensor.matmul(px[:], S_b[k][:, 128 * j:128 * (j + 1)],
                                     xc[k],
                                     start=(k == 0), stop=(k == NK - 1))
                xs = xtpool.tile([128, 128], BF16, tag="xs", name="xs")
                nc.scalar.copy(xs[:], px[:])
                xn.append(xs)
            xp, xc = xc, [t[:] for t in xn]
        pnum = ps_m.tile([1, 128], F32, tag="pps", name="pps")
        pden = ps_m.tile([1, 128], F32, tag="pps", name="pps")
        for k in range(NK):
            prod_n = xtpool.tile([128, 128], F32, tag="prodn", name="prodn")
            prod_d = xtpool.tile([128, 128], F32, tag="prodd", name="prodd")
            nc.vector.tensor_tensor(prod_n[:], xc[k], xc[k], OP.mult)
            nc.vector.tensor_tensor(prod_d[:], xp[k], xc[k], OP.mult)
            nc.tensor.matmul(pnum[:], ONES[:], prod_n[:],
                             start=(k == 0), stop=(k == NK - 1))
            nc.tensor.matmul(pden[:], ONES[:], prod_d[:],
                             start=(k == 0), stop=(k == NK - 1))
        nc.vector.reciprocal(ray_i[:], pden[:])
        nc.vector.tensor_tensor(ray[:], pnum[:], ray_i[:], OP.mult)
        nc.vector.tensor_reduce(lmax[:], ray[:], axis=mybir.AxisListType.X, op=OP.max)
        nc.vector.tensor_scalar(lsafe[:], lmax[:], float(safety), None, OP.mult)
        nc.vector.reciprocal(lr_s[:], lsafe[:])
        nc.vector.tensor_scalar(nlr_s[:], lr_s[:], -1.0, None, OP.mult)
        nc.gpsimd.partition_broadcast(lr_vec[:], lr_s[:])
        nc.gpsimd.partition_broadcast(nlr_vec[:], nlr_s[:])

        # ---- A = I - lr*Sigma (+casts);  C = -lr*p (f32r, exact fp32 bits) ----
        for k in range(NK):
            nc.vector.scalar_tensor_tensor(A[k][:], S[k][:], nlr_vec[:, 0:1],
                                           IA[k][:], op0=OP.mult, op1=OP.add)
            nc.vector.tensor_copy(A_r[k][:], A[k][:])
            nc.vector.tensor_copy(A_b[k][:], A[k][:])
        for b in range(NB):
            nc.vector.tensor_scalar(C_r[b][:], P[b][:], nlr_vec[:, 0:1], None,
                                    OP.mult)

        # ---- initial weights: w0 = 1/N (transpose-invariant) ----
        wta = []
        for b in range(NB):
            a0 = wtpool.tile([128, N], mdt(0), tag=f"wta{b}", name=f"wta{b}")
            nc.gpsimd.memset(a0[:], 1.0 / N)
            wta.append(a0)

        w_cur = [None] * NB      # w(t) tiles (for momentum diff)
        y_cur = [None] * NB      # y'(t) tiles (transposed in next round)

        def emit_mms(b, t, pw):
            Amm = {BF16: A_b, F32R: A_r, F32: A}[mdt(t)]
            for k in range(NK):
                nc.tensor.matmul(pw[:], wta[b][:, 128 * k:128 * (k + 1)],
                                 Amm[k][:], start=(k == 0), stop=False)
            nc.tensor.matmul(pw[:], ID_r[:], C_r[b][:], start=False, stop=True)

        def emit_h2(b, t):
            """transpose+copy y'(t) -> wta for step t+1."""
            dt_n = mdt(t + 1)
            IDmm = {BF16: ID_b, F32R: ID_r, F32: ID}[dt_n]
            y = y_cur[b]
            pt = ps_t.tile([128, N], dt_n, tag="psT", name="psT")
            for k in range(NK):
                sl = slice(128 * k, 128 * (k + 1))
                nc.tensor.transpose(pt[:, sl], y[:, sl], IDmm[:])
            nwa = wtpool.tile([128, N], dt_n, tag=f"wta{b}", name=f"wta{b}")
            nc.vector.tensor_copy(nwa[:], pt[:])
            wta[b] = nwa

        def emit_newton(b, t, pw):
            """ACT r-pass + theta Newton update (in-step)."""
            r = rpool.tile([128, N], edt(t), tag=f"r{b}", name=f"r{b}")
            nc.scalar.activation(r[:], pw[:], RELU, bias=th[b], accum_out=sv[b])
            nc.vector.scalar_tensor_tensor(dl[b], sv[b], 1.0, ic[b],
                                           op0=OP.subtract, op1=OP.mult)
            nc.vector.tensor_tensor(th[b], th[b], dl[b], OP.subtract)

        def emit_w(b, t, pw):
            dt_w = edt(t)
            w = wpool.tile([128, N], dt_w, tag=f"w{b}", name=f"w{b}")
            nc.scalar.activation(w[:], pw[:], RELU, bias=th[b])
            w_cur[b] = w
            return w

        def emit_cnt(b, w):
            m = mpool.tile([128, N], BF16, tag=f"m{b}", name=f"m{b}")
            nc.vector.tensor_scalar(m[:], w[:], 0.0, None,
                                    OP.is_gt, OP.add, accum_out=cv[b])
            nc.vector.tensor_scalar(cc[b], cv[b], 1.0, None, OP.max)
            nc.vector.reciprocal(ic[b], cc[b])

        def emit_mom(b, t, w):
            """d = w - w_prev ; y' = beta*d + w  (dtype of step t+1)."""
            dt_n = mdt(t + 1)
            d = dpool.tile([128, N], dt_n, tag=f"d{b}", name=f"d{b}")
            if t == 0:
                nc.vector.tensor_scalar(d[:], w[:], 1.0 / N, None, OP.subtract)
            else:
                nc.vector.tensor_tensor(d[:], w[:], w_cur_prev[b][:], OP.subtract)
            y = ypool.tile([128, N], dt_n, tag=f"y{b}", name=f"y{b}")
            nc.vector.scalar_tensor_tensor(y[:], d[:], beta, w[:],
                                           op0=OP.mult, op1=OP.add)
            y_cur[b] = y

        w_cur_prev = [None] * NB

        # ================= cold start: step 0, both chains =================
        pws = []
        for b in range(NB):
            pw = ps_w.tile([128, N], F32, tag="psW", name="psW")
            emit_mms(b, 0, pw)
            pws.append(pw)
        # th0 = -(sum(z) - 1)/N  (stored negated; bias adds)
        for b in range(NB):
            scr = rpool.tile([128, N], F32, tag=f"r{b}", name=f"r{b}")
            nc.scalar.activation(scr[:], pws[b][:], COPY, accum_out=sv[b])
            nc.vector.tensor_scalar(th[b], sv[b], 1.0, -1.0 / N,
                                    OP.subtract, OP.mult)
        for it in range(k0):
            for b in range(NB):
                r = rpool.tile([128, N], F32, tag=f"r{b}", name=f"r{b}")
                nc.scalar.activation(r[:], pws[b][:], RELU,
                                     bias=th[b], accum_out=sv[b])
                m = mpool.tile([128, N], BF16, tag=f"m{b}", name=f"m{b}")
                nc.vector.tensor_scalar(m[:], r[:], 0.0, None,
                                        OP.is_gt, OP.add, accum_out=cv[b])
            for b in range(NB):
                nc.vector.tensor_scalar(cc[b], cv[b], 1.0, None, OP.max)
                nc.vector.reciprocal(ic[b], cc[b])
                nc.vector.scalar_tensor_tensor(dl[b], sv[b], 1.0, ic[b],
                                               op0=OP.subtract, op1=OP.mult)
                nc.vector.tensor_tensor(th[b], th[b], dl[b], OP.subtract)
        # in-step Newton + w + momentum for step 0
        for b in range(NB):
            emit_newton(b, 0, pws[b])
        for b in range(NB):
            w = emit_w(b, 0, pws[b])
            emit_cnt(b, w)
        for b in range(NB):
            emit_mom(b, 0, w_cur[b])
        w_cur_prev = list(w_cur)

        # ================= steady-state rounds =================
        # chain 1 runs one step behind chain 0, op-interleaved emission.
        def emit_round(t0, t1):
            """chain0 does step t0, chain1 does step t1 (= t0-1); either None."""
            chains = []
            if t1 is not None:
                chains.append((1, t1))
            if t0 is not None:
                chains.append((0, t0))
            pw_map = {}
            for b, t in chains:
                emit_h2(b, t - 1)
            for b, t in chains:
                pw = ps_w.tile([128, N], F32, tag="psW", name="psW")
                emit_mms(b, t, pw)
                pw_map[b] = pw
            for b, t in chains:
                emit_newton(b, t, pw_map[b])
            for b, t in chains:
                w = emit_w(b, t, pw_map[b])
                if t == n_steps - 1:
                    nc.sync.dma_start(out_w[128 * b:128 * (b + 1), :], w[:])
            for b, t in chains:
                if t == n_steps - 1:
                    continue
                if t % CNT_EVERY == 0 or t == n_steps - 2:
                    emit_cnt(b, w_cur[b])
                emit_mom(b, t, w_cur[b])
                w_cur_prev[b] = w_cur[b]

        for t in range(1, n_steps + 1):
            t0 = t if t < n_steps else None
            t1 = t - 1 if t >= 2 else None
            emit_round(t0, t1)


def build_nc(**kw):
    nc = bacc.Bacc("TRN2", target_bir_lowering=False, debug=False,
                   enable_asserts=False)
    p_in = nc.dram_tensor("p", [B_CORE, N], F32, kind="ExternalInput")
    s_in = nc.dram_tensor("sigma", [N, N], F32, kind="ExternalInput")
    w_out = nc.dram_tensor("w", [B_CORE, N], F32, kind="ExternalOutput")
    with tile.TileContext(nc) as tc:
        markowitz_tile_kernel(tc, w_out.ap(), p_in.ap(), s_in.ap(), **kw)
    nc.compile()
    return nc


_NC_CACHE = {}


def kernel(p_batch: np.ndarray, Sigma: np.ndarray, **kw) -> np.ndarray:
    B = p_batch.shape[0]
    rows = B // N_CORES
    assert rows == B_CORE and Sigma.shape == (N, N)
    key = tuple(sorted(kw.items()))
    if key not in _NC_CACHE:
        _NC_CACHE[key] = build_nc(**kw)
    nc = _NC_CACHE[key]
    p32 = np.ascontiguousarray(p_batch, dtype=np.float32)
    s32 = np.ascontiguousarray(Sigma, dtype=np.float32)
    in_maps = [{"p": p32[i * rows:(i + 1) * rows], "sigma": s32}
               for i in range(N_CORES)]
    res = run_bass_kernel_spmd(nc, in_maps, core_ids=list(range(N_CORES)))
    out = np.concatenate([r["w"] for r in res.results], axis=0)
    return out.astype(p_batch.dtype, copy=False)
